# revision 49
# baseline (speedup 1.0000x reference)
"""Trainium2 Bass kernel for nn_LocalCausalGraph.

Math (reference):
    cause  = x @ Wc.T;  effect = x @ We.T            (B, L, cd)
    hc = cause @ W1[:, :cd].T;  he = effect @ W1[:, cd:].T
    h[b,i,j,:] = hc[b,i] + he[b,j] + b1
    out = sigmoid(gelu_exact(h) @ W2.T + b2)          (B, L, L)

Restructure: hc = x @ (W1c @ Wc).T — the chained projections collapse into
one matmul per branch with combined 64x1024 weights (built on device from
bf16 inputs).

Sharding: each of 8 cores owns a 64-row i-slice of the pairwise grid per
batch (needs full `he`, which is tiny, plus its own `hc` slice).

Key layout/scheduling choices (v2):
  * host passes x pre-transposed to (B, D, L) bf16 so every contraction
    (over d) has d on partitions — no on-device transpose anywhere
  * pairwise tiles pack 2 i-rows as 2x64 channels on 128 partitions; the
    broadcast add runs as VectorE 4x-mode tensor_scalar; the exact gelu as
    one ScalarE ACTIVATE per chunk of packed tiles (ACT is the bottleneck
    engine: 65536 free-elems x 0.83ns is ~55us of irreducible work)
  * the combined projection weights Mc=W1c@Wc / Me=W1e@We are folded on
    the host (pure weight preprocessing) and shipped with DUPLICATED
    column pairs (128-wide stationaries), so one matmul pass fills both
    PSUM partition halves — half the projection matmuls of the 2-pass
    variant at identical PE cost per pass, and no weight-combine chain
    on the device's critical path
  * batch 0 is j-split: DMA of x[0] lands in j-halves and he/adds/gelu/
    score all run on j-quarters, so the first gelu fires ~5us in instead
    of waiting for the full he row
  * 8-tile gelu chunks; next-batch projections are emitted between a
    chunk's gelu and its score matmuls, and each batch's tanh is deferred
    past the next batch's first gelu — PE never idles >3us (which would
    drop it to the slow pstate) and ACT stays saturated
  * all gelus precede all sigmoids; sigmoid(x) = 0.5 + 0.5*tanh(0.5x +
    0.5*b2) reuses the gelu ACT table set, so the tail pays no table switch
"""

import os
import numpy as np
import ml_dtypes

import concourse.bass as bass
import concourse.bacc as bacc
import concourse.mybir as mybir
import concourse.tile as tile

FP32 = mybir.dt.float32
BF16 = mybir.dt.bfloat16
AF = mybir.ActivationFunctionType

B, L, D, CD = 4, 512, 1024, 64
N_CORES = 8
IC = L // N_CORES          # i-rows per core per batch = 64
NT = IC // 2               # packed (2-row) tiles per batch = 32
DT = D // 128              # contraction d-tiles = 8

# chunk plans per batch: (t0, t1, j0, j1) — packed-tile range x j range.
# b0 runs the first 8 tiles as j-quarters (the first split again in two)
# to shorten the head; the last batch ends on a 2-tile chunk to shorten
# the tail.
PLANS = [
    [(0, 4, 0, 128), (4, 8, 0, 128),
     (0, 8, 128, 256), (0, 8, 256, 384), (0, 8, 384, 512),
     (8, 16, 0, 512), (16, 24, 0, 512), (24, 32, 0, 512)]
    if os.environ.get("KABL0", "") == "" else
    [(0, 8, 0, 512), (8, 16, 0, 512), (16, 24, 0, 512), (24, 32, 0, 512)],
    [(0, 8, 0, 512), (8, 16, 0, 512), (16, 24, 0, 512), (24, 32, 0, 512)],
    [(0, 8, 0, 512), (8, 16, 0, 512), (16, 24, 0, 512), (24, 32, 0, 512)],
    # last batch runs j-halves: the first half's scores close early, so
    # its tanh/sigmoid/DMA-out overlap the second half's gelus; the second
    # half re-chunks small so the final scores hide under later gelus
    [(0, 16, 0, 256), (16, 32, 0, 256),
     (0, 8, 256, 512), (8, 16, 256, 512), (16, 24, 256, 512),
     (24, 30, 256, 512), (30, 32, 256, 512)],
]


def build_kernel(reps: int = 1) -> bass.Bass:
    """reps>1 wraps the whole body in a hardware loop — bench-only mode used
    by the dev harness to amortize dispatch overhead when timing."""
    nc = bacc.Bacc()

    # batches 1..3 of x, pre-transposed to (D, L); batch 0 ships separately
    xt = nc.declare_dram_parameter("xt", [B, D, L], BF16, isOutput=False)
    # x[0] as four contiguous partition-major j-quarters — quarter-sized
    # DMAs with full-width runs (no small-run penalty) that land
    # independently, so batch 0's pipeline starts after ~1.5KB/partition
    x0q = nc.declare_dram_parameter("x0q", [128, 4 * DT * 128], BF16, isOutput=False)
    # xti pre-swizzled on host to partition-major (128, B*DT*IC) so the DMA
    # is one contiguous run per partition
    xti = nc.declare_dram_parameter("xti", [128, B * DT * IC], BF16, isOutput=False)
    # host-folded projection weights: cols 0:1024 = met2 (per d-chunk
    # [Me.T | Me.T] duplicated pairs), cols 1024:1536 = mct (single copy —
    # the hc stationary is only 64 wide)
    mpack = nc.declare_dram_parameter("mpack", [128, DT * 128 + DT * CD], BF16, isOutput=False)
    bpack = nc.declare_dram_parameter("bpack", [128, 2], FP32, isOutput=False)
    w2big = nc.declare_dram_parameter("w2big", [128, NT * CD], BF16, isOutput=False)
    out = nc.declare_dram_parameter("out", [B, IC, L], FP32, isOutput=True)

    import contextlib

    with tile.TileContext(nc) as tc:
        with (
            tc.tile_pool(name="const", bufs=1) as const,
            tc.tile_pool(name="work", bufs=4) as work,
            tc.tile_pool(name="pphe", bufs=4, space="PSUM") as pphe,
            tc.tile_pool(name="phc", bufs=2, space="PSUM") as phc,
            tc.tile_pool(name="psc", bufs=2, space="PSUM") as psc,
            tc.For_i(0, reps, 1) if reps > 1 else contextlib.nullcontext(),
        ):
            # ---- DMAs on one queue, in critical-path priority order.
            # Transfers serialize on the DMA engines, so the first-gelu
            # chain (met2, x0 quarter 0, mct, xti0) leads with ~2.2us of
            # bytes and the remaining x0 quarters follow one per ~0.7us.
            mp_sb = const.tile([128, DT * 128 + DT * CD], BF16)
            nc.sync.dma_start(out=mp_sb[:, 0:DT * 128], in_=mpack[:, 0:DT * 128])
            bp_sb = const.tile([128, 2], FP32)
            nc.sync.dma_start(out=bp_sb, in_=bpack[:, :])
            x0_sb = const.tile([128, 4, DT, 128], BF16)
            nc.sync.dma_start(
                out=x0_sb[:, 0].rearrange("p a b -> p (a b)"),
                in_=x0q[:, 0:DT * 128],
            )
            nc.sync.dma_start(out=mp_sb[:, DT * 128:], in_=mpack[:, DT * 128:])
            xti_sb = const.tile([128, B, DT, IC], BF16)
            nc.sync.dma_start(
                out=xti_sb[:, 0].rearrange("p a b -> p (a b)"),
                in_=xti[:, 0:DT * IC],
            )
            for q in range(1, 4):
                nc.sync.dma_start(
                    out=x0_sb[:, q].rearrange("p a b -> p (a b)"),
                    in_=x0q[:, q * DT * 128:(q + 1) * DT * 128],
                )
            w2_sb = const.tile([128, NT * CD], BF16)
            nc.sync.dma_start(out=w2_sb, in_=w2big[:, :])
            xt_sb = const.tile([128, B, DT, L], BF16)
            nc.sync.dma_start(
                out=xt_sb[:, 1, :, :],
                in_=xt[1].rearrange("(dt p) l -> p dt l", p=128),
            )
            nc.sync.dma_start(
                out=xti_sb[:, 1:4].rearrange("p n a b -> p (n a b)"),
                in_=xti[:, DT * IC:],
            )
            for b in range(2, B):
                nc.sync.dma_start(
                    out=xt_sb[:, b, :, :],
                    in_=xt[b].rearrange("(dt p) l -> p dt l", p=128),
                )

            met2_sb = mp_sb[:, 0:DT * 128]
            mct_sb = mp_sb[:, DT * 128:]
            b1_sb = bp_sb[:, 0:1]
            b2_sb = bp_sb[0:CD, 1:2]

            # ---- PE clock warm-up: a few throwaway matmuls on a zeroed tile
            # so the projection matmuls don't run at the cold pstate (the PE
            # clock needs ~3us of continuous work to reach full speed)
            wu_sb = const.tile([128, 512], BF16)
            nc.vector.memset(wu_sb, 0.0)
            wu_ps = psc.tile([64, 512], FP32, tag="sc", name="wu_ps")
            for _ in range(6):
                nc.tensor.matmul(
                    wu_ps, lhsT=wu_sb[:, 0:64], rhs=wu_sb,
                    start=True, stop=True,
                )

            he_ps, hc_ps = {}, {}
            he2, hc2 = {}, {}

            def he_pe(b, j0, j1):
                # one matmul pass fills BOTH psum partition halves thanks to
                # the duplicated stationary columns. Each j-chunk gets its
                # own PSUM tile: a reader of a psum region waits for the
                # whole tile's accumulation group, so sharing one tile
                # across chunks would serialize the first evacuation behind
                # the last chunk's matmuls.
                # uniform slot shape: mixing tile sizes under one pool tag
                # across slot-reuse generations mis-tracks the WAR deps
                ps = pphe.tile([128, L], FP32, tag="phe",
                               name=f"he_ps_{b}_{j0}")[:, 0:j1 - j0]
                he_ps[(b, j0)] = ps
                for ch in range(DT):
                    rhs = (x0_sb[:, j0 // 128, ch, :]
                           if b == 0 and j1 - j0 == 128
                           else xt_sb[:, b, ch, j0:j1])
                    nc.tensor.matmul(
                        ps,
                        lhsT=met2_sb[:, ch * 128:(ch + 1) * 128],
                        rhs=rhs,
                        start=(ch == 0), stop=(ch == DT - 1),
                    )

            def hc_pe(b):
                # hc packs i-row t with i-row NT+t on the partition halves —
                # different free ranges per half, so two passes with the
                # 64-wide (first duplicate) stationary
                hc_ps[b] = phc.tile([128, NT], FP32, tag="phc", name=f"hc_ps_{b}")
                for half in range(2):
                    for ch in range(DT):
                        nc.tensor.matmul(
                            hc_ps[b][half * CD:(half + 1) * CD, :],
                            lhsT=mct_sb[:, ch * CD:(ch + 1) * CD],
                            rhs=xti_sb[:, b, ch, half * NT:(half + 1) * NT],
                            start=(ch == 0), stop=(ch == DT - 1),
                        )

            def he_dve(b, j0, j1, eng=None):
                # the (otherwise idle) GpSimd engine handles most PSUM
                # evacuations so the in-order DVE stays clear for the
                # latency-critical broadcast adds
                if b not in he2:
                    he2[b] = const.tile([128, L], BF16, name=f"he2_{b}")
                (eng or nc.vector).tensor_scalar_add(
                    he2[b][:, j0:j1], he_ps[(b, j0)], b1_sb
                )

            def hc_dve(b):
                hc2[b] = const.tile([128, NT], FP32, name=f"hc2_{b}")
                nc.vector.tensor_copy(hc2[b], hc_ps[b])

            if os.environ.get("KABL0", "") == "full":
                B0_QUARTERS = [(0, L)]
                nc.sync.dma_start(
                    out=xt_sb[:, 0, :, :],
                    in_=xt[0].rearrange("(dt p) l -> p dt l", p=128),
                )
            else:
                B0_QUARTERS = [(0, 128), (128, 256), (256, 384), (384, 512)]
            he_pe(0, *B0_QUARTERS[0])
            hc_pe(0)
            hc_dve(0)
            he_dve(0, *B0_QUARTERS[0], eng=nc.vector)
            for (j0, j1) in B0_QUARTERS[1:]:
                he_pe(0, j0, j1)
            if os.environ.get("KABL0", "") == "hequart":
                # ablation: quarter he pipeline, full-j chunk plan
                for (j0, j1) in B0_QUARTERS[1:]:
                    he_dve(0, j0, j1, eng=nc.vector)

            sc_ps = {}
            out_sb = const.tile([CD, B * L], FP32)
            pending_tail = []
            pending_region = []

            def emit_tail_region(b, ps, j0, j1):
                # sigmoid(x + b2) = 0.5 + 0.5*tanh(0.5*x + 0.5*b2); tanh is in
                # the same ACT table set as gelu (no switch); bpack col 1
                # already holds 0.5*b2. The affine runs on the slack VectorE.
                th_b = const.tile([CD, j1 - j0], FP32, name=f"th_{b}_{j0}")
                nc.scalar.activation(
                    th_b, ps, AF.Tanh, bias=b2_sb, scale=0.5
                )
                nc.vector.tensor_scalar(
                    out_sb[:, b * L + j0:b * L + j1], th_b, 0.5, 0.5,
                    mybir.AluOpType.mult, mybir.AluOpType.add,
                )
                nc.sync.dma_start(
                    out=out[b, :, j0:j1],
                    in_=out_sb[:, b * L + j0:b * L + j1],
                )

            def emit_tail(b):
                for (ps, j0, j1) in sc_ps[b]:
                    emit_tail_region(b, ps, j0, j1)

            for b in range(B):
                plan = PLANS[b]
                if b < B - 1:
                    sc_ps[b] = [(psc.tile([CD, L], FP32, tag="sc",
                                          name=f"sc_ps_{b}"), 0, L)]
                    if b == 0 and len(plan) > 4:
                        # b0's j-quarter chunks would interleave several
                        # accumulation groups on one PSUM tile, which
                        # miscompiles — zero the tile once and run every
                        # matmul in accumulate mode instead
                        nc.vector.memset(sc_ps[0][0][0], 0.0)
                else:
                    # last batch: j-split score PSUM so the tail's tanh/
                    # sigmoid/DMA for the first half overlaps the second
                    # half's final score matmuls (a PSUM reader waits for
                    # the whole tile's accumulation group)
                    sc_ps[b] = [
                        (psc.tile([CD, L // 2], FP32, tag="sc",
                                  name=f"sc_ps_{b}a"), 0, L // 2),
                        (psc.tile([CD, L // 2], FP32, tag="sc",
                                  name=f"sc_ps_{b}b"), L // 2, L),
                    ]
                for ci, (t0, t1, j0, j1) in enumerate(plan):
                    ntile, jw = t1 - t0, j1 - j0
                    if b == 0 and t0 == 0 and j0 > 0 and jw == 128:
                        # b0 head: evacuate just this j-quarter before its
                        # adds — the first on the latency-critical DVE, the
                        # later two on the idle (but slower) GpSimd so they
                        # don't serialize behind the DVE adds
                        he_dve(0, j0, j1,
                               eng=nc.vector)
                    elif b + 1 < B and ci == len(plan) - 1:
                        # next batch's psum evacuations, late enough on the
                        # in-order DVE not to stall this batch's adds
                        hc_dve(b + 1)
                        he_dve(b + 1, 0, L, eng=nc.vector)
                    h2 = work.tile([128, ntile, jw], BF16, tag="h2")
                    for t in range(t0, t1):
                        nc.vector.tensor_scalar_add(
                            h2[:, t - t0, :], he2[b][:, j0:j1], hc2[b][:, t:t + 1]
                        )
                    nc.scalar.activation(h2, h2, AF.Gelu)
                    # a closed score region's tanh goes after the NEXT gelu
                    # so the in-order ACT never stalls on score matmuls
                    while pending_region:
                        emit_tail_region(b, *pending_region.pop())
                    if ci == 0:
                        # keep PE fed through this batch: next batch's
                        # projections slot between score bursts
                        if b + 1 < B:
                            he_pe(b + 1, 0, L)
                            hc_pe(b + 1)
                        # previous batch's tanh — its scores are long done
                        while pending_tail:
                            emit_tail(pending_tail.pop())
                    for t in range(t0, t1):
                        for (ps, r0, r1) in sc_ps[b]:
                            lo, hi = max(j0, r0), min(j1, r1)
                            if lo >= hi:
                                continue
                            zeroed = b == 0 and len(plan) > 4
                            nc.tensor.matmul(
                                ps[:, lo - r0:hi - r0],
                                lhsT=w2_sb[:, t * CD:(t + 1) * CD],
                                rhs=h2[:, t - t0, lo - j0:hi - j0],
                                start=False if zeroed else (t == 0),
                                stop=(t == NT - 1),
                                skip_group_check=zeroed,
                            )
                    if b == B - 1 and t1 == NT:
                        # a region of the last batch just closed — queue its
                        # tanh/sigmoid/DMA to overlap the remaining chunks
                        for (ps, r0, r1) in sc_ps[b]:
                            if j0 <= r0 and r1 <= j1:
                                pending_region.append((ps, r0, r1))
                if b < B - 1:
                    pending_tail.append(b)
            while pending_region:
                emit_tail_region(B - 1, *pending_region.pop())

    nc.finalize()
    return nc


def prep_inputs(x, Wc, We, W1, b1, W2, b2):
    """Host-side layout prep (dtype cast / transpose / slicing only)."""
    bf = ml_dtypes.bfloat16
    xtf = np.ascontiguousarray(x.transpose(0, 2, 1)).astype(bf)   # (B, D, L)

    # fold the chained projections into single 64x1024 weights (pure weight
    # preprocessing): hc = x @ (W1c @ Wc).T, he = x @ (W1e @ We).T.
    # Shipped as per-d-chunk (128, 128) stationaries with the 64 columns
    # duplicated so one matmul pass fills both PSUM partition halves.
    mc = (W1[:, :CD] @ Wc).astype(np.float32)   # (CD, D)
    me = (W1[:, CD:] @ We).astype(np.float32)
    mpack = np.zeros((128, DT * 128 + DT * CD), bf)
    for ch in range(DT):
        blk_e = me[:, ch * 128:(ch + 1) * 128].T.astype(bf)   # (128 d, 64 h)
        blk_c = mc[:, ch * 128:(ch + 1) * 128].T.astype(bf)
        mpack[:, ch * 128:ch * 128 + CD] = blk_e
        mpack[:, ch * 128 + CD:(ch + 1) * 128] = blk_e
        mpack[:, DT * 128 + ch * CD:DT * 128 + (ch + 1) * CD] = blk_c

    bpack = np.zeros((128, 2), np.float32)
    bpack[:, 0] = np.concatenate([b1, b1])
    bpack[:, 1] = 0.5 * b2[0]

    w2big = np.zeros((128, NT, CD), bf)
    for t in range(NT):
        w2big[0:CD, t, t] = W2[0].astype(bf)
        w2big[CD:128, t, NT + t] = W2[0].astype(bf)
    w2big = w2big.reshape(128, NT * CD)

    # x[0] as contiguous partition-major j-quarters
    x0 = xtf[0].reshape(DT, 128, 4, 128)          # (dt, p, q, 128)
    x0q = np.ascontiguousarray(
        x0.transpose(1, 2, 0, 3).reshape(128, 4 * DT * 128)
    )

    shared = {"xt": xtf, "x0q": x0q, "mpack": mpack, "bpack": bpack,
              "w2big": w2big}
    in_maps = []
    for k in range(N_CORES):
        m = dict(shared)
        sl = xtf[:, :, k * IC:(k + 1) * IC].reshape(B, DT, 128, IC)
        m["xti"] = np.ascontiguousarray(
            sl.transpose(2, 0, 1, 3).reshape(128, B * DT * IC)
        )
        in_maps.append(m)
    return in_maps


def kernel(x, Wc, We, W1, b1, W2, b2):
    from concourse.bass_utils import run_bass_kernel_spmd

    x, Wc, We, W1, b1, W2, b2 = (
        np.asarray(a) for a in (x, Wc, We, W1, b1, W2, b2)
    )
    nc = build_kernel()
    in_maps = prep_inputs(x, Wc, We, W1, b1, W2, b2)
    res = run_bass_kernel_spmd(nc, in_maps, list(range(N_CORES)))
    full = np.empty((B, L, L), np.float32)
    for k in range(N_CORES):
        full[:, k * IC:(k + 1) * IC, :] = res.results[k]["out"]
    return full
